# revision 9
# baseline (speedup 1.0000x reference)
"""Trainium2 Bass kernel: 3x3 same-padding Conv2D, NCHW.

Input  (16, 64, 128, 128) f32, weights (128, 64, 3, 3) OIHW, bias (128,).
Output (16, 128, 128, 128) f32.

Strategy: data-parallel over batch — 2 images per NeuronCore on 8 cores.
Per core the conv is computed as accumulated TensorEngine matmuls over
(C_in x tap) contractions:

  - The host pre-builds a padded dual layout per image,
    [128, 130, 130]: partitions 0-63 (copy A) hold the zero-padded image
    shifted down one row (A[r] = padded row r-1), partitions 64-127
    (copy B) hold the padded rows directly (B[r] = padded row r).  One
    fully-contiguous DMA stages it in SBUF (single semaphore wait — the
    LDWEIGHTS half of a self-loading fp32r matmul has very few wait
    slots, so producer count per matmul must stay tiny).
  - For an output row group h..h+3 (free size 4*128 = 512 = one PSUM
    bank) and each kw in 0..2:
      MM1 (K=128): taps (kh=0, kw) on copy A + (kh=1, kw) on copy B in a
      single matmul, since B sits exactly one row below A.
      MM2 (K=64):  tap (kh=2, kw) read from copy A two rows down.
    6 matmuls accumulate into one PSUM bank; epilogue adds bias while
    copying PSUM -> SBUF, then one DMA stores 4 output rows.

Tensors are float32r end-to-end (full-rate fp32 streaming on the PE at
N>=256, vs 4 cycles/row for plain fp32).
"""

import sys

if "/opt/trn_rl_repo" not in sys.path:
    sys.path.insert(0, "/opt/trn_rl_repo")

import numpy as np

N_CORES = 8
IMGS_PER_CORE = 2
H = 128
W = 128
CIN = 64
COUT = 128
WPAD = W + 2  # 130: one zero column each side
HPAD = H + 2  # 130 rows (pad row above and below)
ROWS_PER_GROUP = 4  # 4*128 = 512 free elements = one PSUM bank
WB_COLS = 3 * COUT + 3 * COUT + 1  # w1 (384) | w2 (384, rows 0-63) | bias (1)

# "f32r": fp32 storage, TF32-like matmul (rel err ~2e-4, ~3 PE cycles/row)
# "bf16": bf16 operands via casting DMA (rel err ~3e-3, 1 PE cycle/row + FWL)
DTYPE_MODE = "f16"

_cache = {}


def _build_nc(mode=None):
    import concourse.mybir as mybir
    from concourse import bacc
    from concourse.tile import TileContext

    mode = mode or DTYPE_MODE
    f32 = mybir.dt.float32
    f32r = mybir.dt.float32r
    cdt = {"f32r": f32r, "bf16": mybir.dt.bfloat16, "f16": mybir.dt.float16}[mode]

    nc = bacc.Bacc(target_bir_lowering=False)
    x_d = nc.dram_tensor(
        "x", [IMGS_PER_CORE, 128, HPAD * WPAD], f32r, kind="ExternalInput"
    )
    # packed weights+bias, one DMA:
    #   cols 0..383   : w1[t*64+ci, kw*128+co] = W[co, ci, t, kw], taps kh=t in {0,1}
    #   cols 384..767 : w2[ci, kw*128+co] = W[co, ci, 2, kw] (rows 0..63)
    #   col  768      : bias[co]
    wb_d = nc.dram_tensor("wb", [128, WB_COLS], f32r, kind="ExternalInput")
    out_d = nc.dram_tensor(
        "out", [IMGS_PER_CORE, COUT, H, W], f32, kind="ExternalOutput"
    )

    with TileContext(nc) as tc:
        with (
            tc.tile_pool(name="wpool", bufs=1) as wpool,
            tc.tile_pool(name="xpool", bufs=2) as xpool,
            tc.tile_pool(name="opool", bufs=4) as opool,
            tc.tile_pool(name="pspool", bufs=8, space="PSUM") as pspool,
        ):
            wb_sb = wpool.tile([128, WB_COLS], cdt)
            # gpsimd DMA casts f32 -> bf16 in flight; sync DMA for pure copy
            wdma = nc.gpsimd if cdt != f32r else nc.sync
            wdma.dma_start(out=wb_sb[:], in_=wb_d[:])
            w1_sb = wb_sb[:, 0 : 3 * COUT]
            w2_sb = wb_sb[0:CIN, 3 * COUT : 6 * COUT]
            if mode == "f32r":
                b_sb = wb_sb[:, 6 * COUT : 6 * COUT + 1].bitcast(f32)
            else:
                b_f32 = wpool.tile([COUT, 1], f32)
                nc.sync.dma_start(
                    out=b_f32[:], in_=wb_d[:, 6 * COUT : 6 * COUT + 1].bitcast(f32)
                )
                b_sb = b_f32[:]

            for img in range(IMGS_PER_CORE):
                X = xpool.tile([128, HPAD * WPAD], cdt)
                xdma = nc.gpsimd if cdt != f32r else nc.sync
                xdma.dma_start(out=X[:], in_=x_d[img])
                X3 = X.rearrange("p (r c) -> p r c", c=WPAD)

                for h in range(0, H, ROWS_PER_GROUP):
                    ps = pspool.tile([COUT, ROWS_PER_GROUP * W], f32)
                    for kw in range(3):
                        # taps (kh=0, kw) + (kh=1, kw), K = 128
                        nc.tensor.matmul(
                            ps[:],
                            w1_sb[:, kw * COUT : (kw + 1) * COUT],
                            X3[:, h : h + ROWS_PER_GROUP, kw : kw + W],
                            start=(kw == 0),
                            stop=False,
                        )
                    for kw in range(3):
                        # tap (kh=2, kw), K = 64 on copy A rows h+2..h+5
                        nc.tensor.matmul(
                            ps[:],
                            w2_sb[:, kw * COUT : (kw + 1) * COUT],
                            X3[0:CIN, h + 2 : h + 2 + ROWS_PER_GROUP, kw : kw + W],
                            start=False,
                            stop=(kw == 2),
                        )
                    ob = opool.tile([COUT, ROWS_PER_GROUP * W], f32)
                    # bias-add while evacuating PSUM; alternate engines so
                    # ScalarE and VectorE each carry half the epilogue.
                    if (h // ROWS_PER_GROUP) % 2 == 0:
                        nc.scalar.add(ob[:], ps[:], b_sb)
                    else:
                        nc.vector.tensor_scalar_add(ob[:], ps[:], b_sb)
                    ob3 = ob.rearrange("p (r c) -> p r c", c=W)
                    nc.sync.dma_start(
                        out=out_d[img, :, h : h + ROWS_PER_GROUP, :], in_=ob3[:]
                    )
    nc.compile()
    return nc


def _get_nc(mode=None):
    mode = mode or DTYPE_MODE
    if mode not in _cache:
        _cache[mode] = _build_nc(mode)
    return _cache[mode]


def _make_dual(images):
    """images: [n, 64, 128, 128] -> [n, 128, HPAD*WPAD] dual padded layout."""
    n = images.shape[0]
    zp = np.zeros((n, CIN, HPAD, WPAD), dtype=np.float32)
    zp[:, :, 1 : H + 1, 1 : W + 1] = images  # padded rows 0..129
    dual = np.empty((n, 128, HPAD, WPAD), dtype=np.float32)
    dual[:, 0:CIN] = zp  # A[r] = padded row r-1 shape-wise (row r of zp)
    dual[:, CIN:128, 0 : HPAD - 1] = zp[:, :, 1:HPAD]  # B[r] = padded row r
    dual[:, CIN:128, HPAD - 1] = 0.0  # B row 129 unread
    return np.ascontiguousarray(dual.reshape(n, 128, HPAD * WPAD))


def _prepare_in_maps(input_tensor, weights, bias):
    input_tensor = np.asarray(input_tensor, dtype=np.float32)
    weights = np.asarray(weights, dtype=np.float32)
    bias = np.asarray(bias, dtype=np.float32)
    wb = np.zeros((128, WB_COLS), dtype=np.float32)
    # [co, ci, kh, kw] -> w1[t*64+ci, kw*128+co], w2[ci, kw*128+co]
    wb[:, 0 : 3 * COUT] = (
        weights[:, :, 0:2, :].transpose(2, 1, 3, 0).reshape(128, 3 * COUT)
    )
    wb[0:CIN, 3 * COUT : 6 * COUT] = (
        weights[:, :, 2, :].transpose(1, 2, 0).reshape(CIN, 3 * COUT)
    )
    wb[:, 6 * COUT] = bias
    in_maps = []
    for c in range(N_CORES):
        shard = _make_dual(input_tensor[c * IMGS_PER_CORE : (c + 1) * IMGS_PER_CORE])
        in_maps.append({"x": shard, "wb": wb})
    return in_maps


def _gather(results):
    return np.concatenate([results[c]["out"] for c in range(N_CORES)], axis=0)


def kernel(input_tensor, weights, bias):
    from concourse.bass_utils import run_bass_kernel_spmd

    nc = _get_nc()
    in_maps = _prepare_in_maps(input_tensor, weights, bias)
    res = run_bass_kernel_spmd(nc, in_maps, core_ids=list(range(N_CORES)))
    return _gather(res.results)


# revision 11
# speedup vs baseline: 1.7998x; 1.7998x over previous
"""Trainium2 Bass kernel: 3x3 same-padding Conv2D, NCHW.

Input  (16, 64, 128, 128) f32, weights (128, 64, 3, 3) OIHW, bias (128,).
Output (16, 128, 128, 128) f32.

Strategy: data-parallel over batch — 2 images per NeuronCore on 8 cores.
Per core the conv is computed as accumulated TensorEngine matmuls over
(C_in x tap) contractions:

  - The host pre-builds a padded dual layout per image,
    [128, 130, 130]: partitions 0-63 (copy A) hold the zero-padded image
    shifted down one row (A[r] = padded row r-1), partitions 64-127
    (copy B) hold the padded rows directly (B[r] = padded row r).  One
    fully-contiguous DMA stages it in SBUF (single semaphore wait — the
    LDWEIGHTS half of a self-loading fp32r matmul has very few wait
    slots, so producer count per matmul must stay tiny).
  - For an output row group h..h+3 (free size 4*128 = 512 = one PSUM
    bank) and each kw in 0..2:
      MM1 (K=128): taps (kh=0, kw) on copy A + (kh=1, kw) on copy B in a
      single matmul, since B sits exactly one row below A.
      MM2 (K=64):  tap (kh=2, kw) read from copy A two rows down.
    6 matmuls accumulate into one PSUM bank; epilogue adds bias while
    copying PSUM -> SBUF, then one DMA stores 4 output rows.

Tensors are float32r end-to-end (full-rate fp32 streaming on the PE at
N>=256, vs 4 cycles/row for plain fp32).
"""

import sys

if "/opt/trn_rl_repo" not in sys.path:
    sys.path.insert(0, "/opt/trn_rl_repo")

import numpy as np

N_CORES = 8
IMGS_PER_CORE = 2
H = 128
W = 128
CIN = 64
COUT = 128
WPAD = W + 2  # 130: one zero column each side
HPAD = H + 2  # 130 rows (pad row above and below)
ROWS_PER_GROUP = 4  # 4*128 = 512 free elements = one PSUM bank
WB_COLS = 3 * COUT + 3 * COUT + 1  # w1 (384) | w2 (384, rows 0-63) | bias (1)

# "f32r": fp32 storage, TF32-like matmul (rel err ~2e-4, ~3 PE cycles/row)
# "bf16": bf16 operands via casting DMA (rel err ~3e-3, 1 PE cycle/row + FWL)
DTYPE_MODE = "f16"

_cache = {}


def _build_nc(mode=None):
    import concourse.mybir as mybir
    from concourse import bacc
    from concourse.tile import TileContext

    mode = mode or DTYPE_MODE
    f32 = mybir.dt.float32
    f32r = mybir.dt.float32r
    cdt = {"f32r": f32r, "bf16": mybir.dt.bfloat16, "f16": mybir.dt.float16}[mode]

    nc = bacc.Bacc(target_bir_lowering=False)
    x_d = nc.dram_tensor(
        "x", [IMGS_PER_CORE, 128, HPAD * WPAD], f32r, kind="ExternalInput"
    )
    # packed weights+bias, one DMA:
    #   cols 0..383   : w1[t*64+ci, kw*128+co] = W[co, ci, t, kw], taps kh=t in {0,1}
    #   cols 384..767 : w2[ci, kw*128+co] = W[co, ci, 2, kw] (rows 0..63)
    #   col  768      : bias[co]
    wb_d = nc.dram_tensor("wb", [128, WB_COLS], f32r, kind="ExternalInput")
    out_d = nc.dram_tensor(
        "out", [IMGS_PER_CORE, COUT, H, W], f32, kind="ExternalOutput"
    )

    with TileContext(nc) as tc:
        with (
            tc.tile_pool(name="wpool", bufs=1) as wpool,
            tc.tile_pool(name="xpool", bufs=2) as xpool,
            tc.tile_pool(name="opool", bufs=4) as opool,
            tc.tile_pool(name="pspool", bufs=4, space="PSUM") as pspool,
        ):
            wb_sb = wpool.tile([128, WB_COLS], cdt)
            # gpsimd DMA casts f32 -> bf16 in flight; sync DMA for pure copy
            wdma = nc.gpsimd if cdt != f32r else nc.sync
            wdma.dma_start(out=wb_sb[:], in_=wb_d[:])
            w1_sb = wb_sb[:, 0 : 3 * COUT]
            w2_sb = wb_sb[0:CIN, 3 * COUT : 6 * COUT]
            w2b_sb = wb_sb[CIN:128, 3 * COUT : 6 * COUT]
            if mode == "f32r":
                b_sb = wb_sb[:, 6 * COUT : 6 * COUT + 1].bitcast(f32)
            else:
                b_f32 = wpool.tile([COUT, 1], f32)
                nc.sync.dma_start(
                    out=b_f32[:], in_=wb_d[:, 6 * COUT : 6 * COUT + 1].bitcast(f32)
                )
                b_sb = b_f32[:]

            for img in range(IMGS_PER_CORE):
                X = xpool.tile([128, HPAD * WPAD], cdt)
                xdma = nc.gpsimd if cdt != f32r else nc.sync
                xdma.dma_start(out=X[:], in_=x_d[img])
                X3 = X.rearrange("p (r c) -> p r c", c=WPAD)

                # Supergroups of 8 output rows: two PSUM banks (g: rows
                # h..h+3, g2: rows h+4..h+7).  The kh=2 taps of g and g2
                # are issued as adjacent K=64 matmuls on disjoint
                # partition halves (A rows for g, B rows for g2) -> the
                # PE runs them concurrently, so a supergroup costs 9
                # matmul slots instead of 12 (the K=128 ideal is 9).
                for h in range(0, H, 2 * ROWS_PER_GROUP):
                    ps = pspool.tile([COUT, ROWS_PER_GROUP * W], f32, tag="psA")
                    ps2 = pspool.tile([COUT, ROWS_PER_GROUP * W], f32, tag="psB")
                    for kw in range(3):
                        # g: taps (kh=0, kw) on A + (kh=1, kw) on B, K=128
                        nc.tensor.matmul(
                            ps[:],
                            w1_sb[:, kw * COUT : (kw + 1) * COUT],
                            X3[:, h : h + ROWS_PER_GROUP, kw : kw + W],
                            start=(kw == 0),
                            stop=False,
                        )
                    for kw in range(3):
                        # g2: same, rows h+4..h+7
                        nc.tensor.matmul(
                            ps2[:],
                            w1_sb[:, kw * COUT : (kw + 1) * COUT],
                            X3[:, h + 4 : h + 4 + ROWS_PER_GROUP, kw : kw + W],
                            start=(kw == 0),
                            stop=False,
                        )
                    for kw in range(3):
                        # paired kh=2 taps: g from copy A (partitions 0-63),
                        # g2 from copy B (partitions 64-127) — concurrent.
                        nc.tensor.matmul(
                            ps[:],
                            w2_sb[:, kw * COUT : (kw + 1) * COUT],
                            X3[0:CIN, h + 2 : h + 2 + ROWS_PER_GROUP, kw : kw + W],
                            start=False,
                            stop=(kw == 2),
                        )
                        nc.tensor.matmul(
                            ps2[:],
                            w2b_sb[:, kw * COUT : (kw + 1) * COUT],
                            X3[CIN:128, h + 5 : h + 5 + ROWS_PER_GROUP, kw : kw + W],
                            start=False,
                            stop=(kw == 2),
                        )
                    for half, (pst, hh) in enumerate(((ps, h), (ps2, h + 4))):
                        ob = opool.tile([COUT, ROWS_PER_GROUP * W], f32)
                        # bias-add while evacuating PSUM; alternate engines
                        # so ScalarE and VectorE each carry half of it.
                        if half == 0:
                            nc.scalar.add(ob[:], pst[:], b_sb)
                        else:
                            nc.vector.tensor_scalar_add(ob[:], pst[:], b_sb)
                        ob3 = ob.rearrange("p (r c) -> p r c", c=W)
                        nc.sync.dma_start(
                            out=out_d[img, :, hh : hh + ROWS_PER_GROUP, :], in_=ob3[:]
                        )
    nc.compile()
    return nc


def _get_nc(mode=None):
    mode = mode or DTYPE_MODE
    if mode not in _cache:
        _cache[mode] = _build_nc(mode)
    return _cache[mode]


def _make_dual(images):
    """images: [n, 64, 128, 128] -> [n, 128, HPAD*WPAD] dual padded layout."""
    n = images.shape[0]
    zp = np.zeros((n, CIN, HPAD, WPAD), dtype=np.float32)
    zp[:, :, 1 : H + 1, 1 : W + 1] = images  # padded rows 0..129
    dual = np.empty((n, 128, HPAD, WPAD), dtype=np.float32)
    dual[:, 0:CIN] = zp  # A[r] = padded row r-1 shape-wise (row r of zp)
    dual[:, CIN:128, 0 : HPAD - 1] = zp[:, :, 1:HPAD]  # B[r] = padded row r
    dual[:, CIN:128, HPAD - 1] = 0.0  # B row 129 unread
    return np.ascontiguousarray(dual.reshape(n, 128, HPAD * WPAD))


def _prepare_in_maps(input_tensor, weights, bias):
    input_tensor = np.asarray(input_tensor, dtype=np.float32)
    weights = np.asarray(weights, dtype=np.float32)
    bias = np.asarray(bias, dtype=np.float32)
    wb = np.zeros((128, WB_COLS), dtype=np.float32)
    # [co, ci, kh, kw] -> w1[t*64+ci, kw*128+co], w2[ci, kw*128+co]
    wb[:, 0 : 3 * COUT] = (
        weights[:, :, 0:2, :].transpose(2, 1, 3, 0).reshape(128, 3 * COUT)
    )
    w2 = weights[:, :, 2, :].transpose(1, 2, 0).reshape(CIN, 3 * COUT)
    wb[0:CIN, 3 * COUT : 6 * COUT] = w2
    wb[CIN:128, 3 * COUT : 6 * COUT] = w2  # duplicate for partition-64 row tiles
    wb[:, 6 * COUT] = bias
    in_maps = []
    for c in range(N_CORES):
        shard = _make_dual(input_tensor[c * IMGS_PER_CORE : (c + 1) * IMGS_PER_CORE])
        in_maps.append({"x": shard, "wb": wb})
    return in_maps


def _gather(results):
    return np.concatenate([results[c]["out"] for c in range(N_CORES)], axis=0)


def kernel(input_tensor, weights, bias):
    from concourse.bass_utils import run_bass_kernel_spmd

    nc = _get_nc()
    in_maps = _prepare_in_maps(input_tensor, weights, bias)
    res = run_bass_kernel_spmd(nc, in_maps, core_ids=list(range(N_CORES)))
    return _gather(res.results)


# revision 12
# speedup vs baseline: 2.1185x; 1.1770x over previous
"""Trainium2 Bass kernel: 3x3 same-padding Conv2D, NCHW.

Input  (16, 64, 128, 128) f32, weights (128, 64, 3, 3) OIHW, bias (128,).
Output (16, 128, 128, 128) f32.

Strategy: data-parallel over batch — 2 images per NeuronCore on 8 cores.
Per core the conv is computed as accumulated TensorEngine matmuls over
(C_in x tap) contractions:

  - The host pre-builds a padded dual layout per image,
    [128, 130, 130]: partitions 0-63 (copy A) hold the zero-padded image
    shifted down one row (A[r] = padded row r-1), partitions 64-127
    (copy B) hold the padded rows directly (B[r] = padded row r).  One
    fully-contiguous DMA stages it in SBUF (single semaphore wait — the
    LDWEIGHTS half of a self-loading fp32r matmul has very few wait
    slots, so producer count per matmul must stay tiny).
  - For an output row group h..h+3 (free size 4*128 = 512 = one PSUM
    bank) and each kw in 0..2:
      MM1 (K=128): taps (kh=0, kw) on copy A + (kh=1, kw) on copy B in a
      single matmul, since B sits exactly one row below A.
      MM2 (K=64):  tap (kh=2, kw) read from copy A two rows down.
    6 matmuls accumulate into one PSUM bank; epilogue adds bias while
    copying PSUM -> SBUF, then one DMA stores 4 output rows.

Tensors are float32r end-to-end (full-rate fp32 streaming on the PE at
N>=256, vs 4 cycles/row for plain fp32).
"""

import sys

if "/opt/trn_rl_repo" not in sys.path:
    sys.path.insert(0, "/opt/trn_rl_repo")

import numpy as np

N_CORES = 8
IMGS_PER_CORE = 2
H = 128
W = 128
CIN = 64
COUT = 128
WPAD = W + 2  # 130: one zero column each side
HPAD = H + 2  # 130 rows (pad row above and below)
ROWS_PER_GROUP = 4  # 4*128 = 512 free elements = one PSUM bank
WB_COLS = 3 * COUT + 3 * COUT + 1  # w1 (384) | w2 (384, rows 0-63) | bias (1)

# "f32r": fp32 storage, TF32-like matmul (rel err ~2e-4, ~3 PE cycles/row)
# "bf16": bf16 operands via casting DMA (rel err ~3e-3, 1 PE cycle/row + FWL)
DTYPE_MODE = "f16"

_cache = {}


def _build_nc(mode=None):
    import concourse.mybir as mybir
    from concourse import bacc
    from concourse.tile import TileContext

    mode = mode or DTYPE_MODE
    f32 = mybir.dt.float32
    f32r = mybir.dt.float32r
    # fp16 operands are cast host-side: input DMA traffic halves and the
    # slow SWDGE casting-DMA path (gpsimd descgen + drains) disappears.
    cdt = {"f32r": f32r, "f16": mybir.dt.float16}[mode]

    nc = bacc.Bacc(target_bir_lowering=False)
    x_d = nc.dram_tensor(
        "x", [IMGS_PER_CORE, 128, HPAD * WPAD], cdt, kind="ExternalInput"
    )
    # packed weights+bias, one DMA:
    #   cols 0..383   : w1[t*64+ci, kw*128+co] = W[co, ci, t, kw], taps kh=t in {0,1}
    #   cols 384..767 : w2[ci, kw*128+co] = W[co, ci, 2, kw] (rows 0..63)
    #   col  768      : bias[co]
    wb_d = nc.dram_tensor("wb", [128, 6 * COUT], cdt, kind="ExternalInput")
    b_d = nc.dram_tensor("b", [COUT, 1], f32, kind="ExternalInput")
    out_d = nc.dram_tensor(
        "out", [IMGS_PER_CORE, COUT, H, W], f32, kind="ExternalOutput"
    )

    with TileContext(nc) as tc:
        with (
            tc.tile_pool(name="wpool", bufs=1) as wpool,
            tc.tile_pool(name="xpool", bufs=2) as xpool,
            tc.tile_pool(name="opool", bufs=4) as opool,
            tc.tile_pool(name="pspool", bufs=4, space="PSUM") as pspool,
        ):
            wb_sb = wpool.tile([128, 6 * COUT], cdt)
            nc.sync.dma_start(out=wb_sb[:], in_=wb_d[:])
            w1_sb = wb_sb[:, 0 : 3 * COUT]
            w2_sb = wb_sb[0:CIN, 3 * COUT : 6 * COUT]
            w2b_sb = wb_sb[CIN:128, 3 * COUT : 6 * COUT]
            b_f32 = wpool.tile([COUT, 1], f32)
            nc.sync.dma_start(out=b_f32[:], in_=b_d[:])
            b_sb = b_f32[:]

            for img in range(IMGS_PER_CORE):
                X = xpool.tile([128, HPAD * WPAD], cdt)
                nc.sync.dma_start(out=X[:], in_=x_d[img])
                X3 = X.rearrange("p (r c) -> p r c", c=WPAD)

                # Supergroups of 8 output rows: two PSUM banks (g: rows
                # h..h+3, g2: rows h+4..h+7).  The kh=2 taps of g and g2
                # are issued as adjacent K=64 matmuls on disjoint
                # partition halves (A rows for g, B rows for g2) -> the
                # PE runs them concurrently, so a supergroup costs 9
                # matmul slots instead of 12 (the K=128 ideal is 9).
                for h in range(0, H, 2 * ROWS_PER_GROUP):
                    ps = pspool.tile([COUT, ROWS_PER_GROUP * W], f32, tag="psA")
                    ps2 = pspool.tile([COUT, ROWS_PER_GROUP * W], f32, tag="psB")
                    for kw in range(3):
                        # g: taps (kh=0, kw) on A + (kh=1, kw) on B, K=128
                        nc.tensor.matmul(
                            ps[:],
                            w1_sb[:, kw * COUT : (kw + 1) * COUT],
                            X3[:, h : h + ROWS_PER_GROUP, kw : kw + W],
                            start=(kw == 0),
                            stop=False,
                        )
                    for kw in range(3):
                        # g2: same, rows h+4..h+7
                        nc.tensor.matmul(
                            ps2[:],
                            w1_sb[:, kw * COUT : (kw + 1) * COUT],
                            X3[:, h + 4 : h + 4 + ROWS_PER_GROUP, kw : kw + W],
                            start=(kw == 0),
                            stop=False,
                        )
                    for kw in range(3):
                        # paired kh=2 taps: g from copy A (partitions 0-63),
                        # g2 from copy B (partitions 64-127) — concurrent.
                        nc.tensor.matmul(
                            ps[:],
                            w2_sb[:, kw * COUT : (kw + 1) * COUT],
                            X3[0:CIN, h + 2 : h + 2 + ROWS_PER_GROUP, kw : kw + W],
                            start=False,
                            stop=(kw == 2),
                        )
                        nc.tensor.matmul(
                            ps2[:],
                            w2b_sb[:, kw * COUT : (kw + 1) * COUT],
                            X3[CIN:128, h + 5 : h + 5 + ROWS_PER_GROUP, kw : kw + W],
                            start=False,
                            stop=(kw == 2),
                        )
                    for half, (pst, hh) in enumerate(((ps, h), (ps2, h + 4))):
                        ob = opool.tile([COUT, ROWS_PER_GROUP * W], f32)
                        # bias-add while evacuating PSUM; alternate engines
                        # so ScalarE and VectorE each carry half of it.
                        if half == 0:
                            nc.scalar.add(ob[:], pst[:], b_sb)
                        else:
                            nc.vector.tensor_scalar_add(ob[:], pst[:], b_sb)
                        ob3 = ob.rearrange("p (r c) -> p r c", c=W)
                        nc.sync.dma_start(
                            out=out_d[img, :, hh : hh + ROWS_PER_GROUP, :], in_=ob3[:]
                        )
    nc.compile()
    return nc


def _get_nc(mode=None):
    mode = mode or DTYPE_MODE
    if mode not in _cache:
        _cache[mode] = _build_nc(mode)
    return _cache[mode]


def _make_dual(images):
    """images: [n, 64, 128, 128] -> [n, 128, HPAD*WPAD] dual padded layout."""
    n = images.shape[0]
    zp = np.zeros((n, CIN, HPAD, WPAD), dtype=np.float32)
    zp[:, :, 1 : H + 1, 1 : W + 1] = images  # padded rows 0..129
    dual = np.empty((n, 128, HPAD, WPAD), dtype=np.float32)
    dual[:, 0:CIN] = zp  # A[r] = padded row r-1 shape-wise (row r of zp)
    dual[:, CIN:128, 0 : HPAD - 1] = zp[:, :, 1:HPAD]  # B[r] = padded row r
    dual[:, CIN:128, HPAD - 1] = 0.0  # B row 129 unread
    return np.ascontiguousarray(dual.reshape(n, 128, HPAD * WPAD))


def _prepare_in_maps(input_tensor, weights, bias, mode=None):
    mode = mode or DTYPE_MODE
    hdt = np.float32 if mode == "f32r" else np.float16
    input_tensor = np.asarray(input_tensor, dtype=np.float32)
    weights = np.asarray(weights, dtype=np.float32)
    bias = np.asarray(bias, dtype=np.float32)
    wb = np.zeros((128, 6 * COUT), dtype=np.float32)
    # [co, ci, kh, kw] -> w1[t*64+ci, kw*128+co], w2[ci, kw*128+co]
    wb[:, 0 : 3 * COUT] = (
        weights[:, :, 0:2, :].transpose(2, 1, 3, 0).reshape(128, 3 * COUT)
    )
    w2 = weights[:, :, 2, :].transpose(1, 2, 0).reshape(CIN, 3 * COUT)
    wb[0:CIN, 3 * COUT : 6 * COUT] = w2
    wb[CIN:128, 3 * COUT : 6 * COUT] = w2  # duplicate for partition-64 row tiles
    wb = np.ascontiguousarray(wb.astype(hdt))
    b = np.ascontiguousarray(bias.reshape(COUT, 1))
    in_maps = []
    for c in range(N_CORES):
        shard = _make_dual(
            input_tensor[c * IMGS_PER_CORE : (c + 1) * IMGS_PER_CORE]
        ).astype(hdt)
        in_maps.append({"x": shard, "wb": wb, "b": b})
    return in_maps


def _gather(results):
    return np.concatenate([results[c]["out"] for c in range(N_CORES)], axis=0)


def kernel(input_tensor, weights, bias):
    from concourse.bass_utils import run_bass_kernel_spmd

    nc = _get_nc()
    in_maps = _prepare_in_maps(input_tensor, weights, bias)
    res = run_bass_kernel_spmd(nc, in_maps, core_ids=list(range(N_CORES)))
    return _gather(res.results)


# revision 13
# speedup vs baseline: 2.4146x; 1.1398x over previous
"""Trainium2 Bass kernel: 3x3 same-padding Conv2D, NCHW.

Input  (16, 64, 128, 128) f32, weights (128, 64, 3, 3) OIHW, bias (128,).
Output (16, 128, 128, 128) f32.

Strategy: data-parallel over batch — 2 images per NeuronCore on 8 cores.
Per core the conv is computed as accumulated TensorEngine matmuls over
(C_in x tap) contractions:

  - The host pre-builds a padded dual layout per image,
    [128, 130, 130]: partitions 0-63 (copy A) hold the zero-padded image
    shifted down one row (A[r] = padded row r-1), partitions 64-127
    (copy B) hold the padded rows directly (B[r] = padded row r).  One
    fully-contiguous DMA stages it in SBUF (single semaphore wait — the
    LDWEIGHTS half of a self-loading fp32r matmul has very few wait
    slots, so producer count per matmul must stay tiny).
  - For an output row group h..h+3 (free size 4*128 = 512 = one PSUM
    bank) and each kw in 0..2:
      MM1 (K=128): taps (kh=0, kw) on copy A + (kh=1, kw) on copy B in a
      single matmul, since B sits exactly one row below A.
      MM2 (K=64):  tap (kh=2, kw) read from copy A two rows down.
    6 matmuls accumulate into one PSUM bank; epilogue adds bias while
    copying PSUM -> SBUF, then one DMA stores 4 output rows.

Tensors are float32r end-to-end (full-rate fp32 streaming on the PE at
N>=256, vs 4 cycles/row for plain fp32).
"""

import sys

if "/opt/trn_rl_repo" not in sys.path:
    sys.path.insert(0, "/opt/trn_rl_repo")

import numpy as np

N_CORES = 8
IMGS_PER_CORE = 2
H = 128
W = 128
CIN = 64
COUT = 128
WPAD = W + 2  # 130: one zero column each side
HPAD = H + 2  # 130 rows (pad row above and below)
ROWS_PER_GROUP = 4  # 4*128 = 512 free elements = one PSUM bank
WB_COLS = 3 * COUT + 3 * COUT + 1  # w1 (384) | w2 (384, rows 0-63) | bias (1)

# "f32r": fp32 storage, TF32-like matmul (rel err ~2e-4, ~3 PE cycles/row)
# "bf16": bf16 operands via casting DMA (rel err ~3e-3, 1 PE cycle/row + FWL)
DTYPE_MODE = "f16"

_cache = {}


def _build_nc(mode=None):
    import concourse.mybir as mybir
    from concourse import bacc
    from concourse.tile import TileContext

    mode = mode or DTYPE_MODE
    f32 = mybir.dt.float32
    f32r = mybir.dt.float32r
    # fp16 operands are cast host-side: input DMA traffic halves and the
    # slow SWDGE casting-DMA path (gpsimd descgen + drains) disappears.
    cdt = {"f32r": f32r, "f16": mybir.dt.float16}[mode]

    nc = bacc.Bacc(target_bir_lowering=False)
    x_d = nc.dram_tensor(
        "x", [IMGS_PER_CORE, 128, HPAD * WPAD], cdt, kind="ExternalInput"
    )
    # packed weights+bias, one DMA:
    #   cols 0..383   : w1[t*64+ci, kw*128+co] = W[co, ci, t, kw], taps kh=t in {0,1}
    #   cols 384..767 : w2[ci, kw*128+co] = W[co, ci, 2, kw] (rows 0..63)
    #   col  768      : bias[co]
    wb_d = nc.dram_tensor("wb", [128, 6 * COUT], cdt, kind="ExternalInput")
    b_d = nc.dram_tensor("b", [COUT, 1], f32, kind="ExternalInput")
    out_d = nc.dram_tensor(
        "out", [IMGS_PER_CORE, COUT, H, W], f32, kind="ExternalOutput"
    )

    with TileContext(nc) as tc:
        with (
            tc.tile_pool(name="wpool", bufs=1) as wpool,
            tc.tile_pool(name="xpool", bufs=2) as xpool,
            tc.tile_pool(name="opool", bufs=4) as opool,
            tc.tile_pool(name="pspool", bufs=4, space="PSUM") as pspool,
        ):
            wb_sb = wpool.tile([128, 6 * COUT], cdt)
            nc.sync.dma_start(out=wb_sb[:], in_=wb_d[:])
            w1_sb = wb_sb[:, 0 : 3 * COUT]
            w2_sb = wb_sb[0:CIN, 3 * COUT : 6 * COUT]
            w2b_sb = wb_sb[CIN:128, 3 * COUT : 6 * COUT]
            b_f32 = wpool.tile([COUT, 1], f32)
            nc.sync.dma_start(out=b_f32[:], in_=b_d[:])
            b_sb = b_f32[:]

            # row-chunk edges for the staged input DMA: compute on the
            # first supergroup starts as soon as chunk 0 (17 rows) lands
            # instead of waiting for the whole 4.2MB image.
            edges = [0, 17, 34, 51, 68, 85, 102, 119, HPAD]
            for img in range(IMGS_PER_CORE):
                X = xpool.tile([128, HPAD * WPAD], cdt)
                for r0, r1 in zip(edges[:-1], edges[1:]):
                    nc.sync.dma_start(
                        out=X[:, r0 * WPAD : r1 * WPAD],
                        in_=x_d[img, :, r0 * WPAD : r1 * WPAD],
                    )
                X3 = X.rearrange("p (r c) -> p r c", c=WPAD)

                # Supergroups of 8 output rows: two PSUM banks (g: rows
                # h..h+3, g2: rows h+4..h+7).  The kh=2 taps of g and g2
                # are issued as adjacent K=64 matmuls on disjoint
                # partition halves (A rows for g, B rows for g2) -> the
                # PE runs them concurrently, so a supergroup costs 9
                # matmul slots instead of 12 (the K=128 ideal is 9).
                for h in range(0, H, 2 * ROWS_PER_GROUP):
                    ps = pspool.tile([COUT, ROWS_PER_GROUP * W], f32, tag="psA")
                    ps2 = pspool.tile([COUT, ROWS_PER_GROUP * W], f32, tag="psB")
                    for kw in range(3):
                        # g: taps (kh=0, kw) on A + (kh=1, kw) on B, K=128
                        nc.tensor.matmul(
                            ps[:],
                            w1_sb[:, kw * COUT : (kw + 1) * COUT],
                            X3[:, h : h + ROWS_PER_GROUP, kw : kw + W],
                            start=(kw == 0),
                            stop=False,
                        )
                    for kw in range(3):
                        # g2: same, rows h+4..h+7
                        nc.tensor.matmul(
                            ps2[:],
                            w1_sb[:, kw * COUT : (kw + 1) * COUT],
                            X3[:, h + 4 : h + 4 + ROWS_PER_GROUP, kw : kw + W],
                            start=(kw == 0),
                            stop=False,
                        )
                    for kw in range(3):
                        # paired kh=2 taps: g from copy A (partitions 0-63),
                        # g2 from copy B (partitions 64-127) — concurrent.
                        nc.tensor.matmul(
                            ps[:],
                            w2_sb[:, kw * COUT : (kw + 1) * COUT],
                            X3[0:CIN, h + 2 : h + 2 + ROWS_PER_GROUP, kw : kw + W],
                            start=False,
                            stop=(kw == 2),
                        )
                        nc.tensor.matmul(
                            ps2[:],
                            w2b_sb[:, kw * COUT : (kw + 1) * COUT],
                            X3[CIN:128, h + 5 : h + 5 + ROWS_PER_GROUP, kw : kw + W],
                            start=False,
                            stop=(kw == 2),
                        )
                    # bias-add while evacuating PSUM into one 8-row tile;
                    # ScalarE and VectorE each carry half.  The single
                    # out-DMA goes on the scalar HWDGE ring so stores never
                    # queue behind the next image's input chunks on sync.
                    ob = opool.tile([COUT, 2 * ROWS_PER_GROUP * W], f32)
                    nc.scalar.add(ob[:, 0 : ROWS_PER_GROUP * W], ps[:], b_sb)
                    nc.vector.tensor_scalar_add(
                        ob[:, ROWS_PER_GROUP * W :], ps2[:], b_sb
                    )
                    ob3 = ob.rearrange("p (r c) -> p r c", c=W)
                    nc.scalar.dma_start(
                        out=out_d[img, :, h : h + 2 * ROWS_PER_GROUP, :], in_=ob3[:]
                    )
    nc.compile()
    return nc


def _get_nc(mode=None):
    mode = mode or DTYPE_MODE
    if mode not in _cache:
        _cache[mode] = _build_nc(mode)
    return _cache[mode]


def _make_dual(images):
    """images: [n, 64, 128, 128] -> [n, 128, HPAD*WPAD] dual padded layout."""
    n = images.shape[0]
    zp = np.zeros((n, CIN, HPAD, WPAD), dtype=np.float32)
    zp[:, :, 1 : H + 1, 1 : W + 1] = images  # padded rows 0..129
    dual = np.empty((n, 128, HPAD, WPAD), dtype=np.float32)
    dual[:, 0:CIN] = zp  # A[r] = padded row r-1 shape-wise (row r of zp)
    dual[:, CIN:128, 0 : HPAD - 1] = zp[:, :, 1:HPAD]  # B[r] = padded row r
    dual[:, CIN:128, HPAD - 1] = 0.0  # B row 129 unread
    return np.ascontiguousarray(dual.reshape(n, 128, HPAD * WPAD))


def _prepare_in_maps(input_tensor, weights, bias, mode=None):
    mode = mode or DTYPE_MODE
    hdt = np.float32 if mode == "f32r" else np.float16
    input_tensor = np.asarray(input_tensor, dtype=np.float32)
    weights = np.asarray(weights, dtype=np.float32)
    bias = np.asarray(bias, dtype=np.float32)
    wb = np.zeros((128, 6 * COUT), dtype=np.float32)
    # [co, ci, kh, kw] -> w1[t*64+ci, kw*128+co], w2[ci, kw*128+co]
    wb[:, 0 : 3 * COUT] = (
        weights[:, :, 0:2, :].transpose(2, 1, 3, 0).reshape(128, 3 * COUT)
    )
    w2 = weights[:, :, 2, :].transpose(1, 2, 0).reshape(CIN, 3 * COUT)
    wb[0:CIN, 3 * COUT : 6 * COUT] = w2
    wb[CIN:128, 3 * COUT : 6 * COUT] = w2  # duplicate for partition-64 row tiles
    wb = np.ascontiguousarray(wb.astype(hdt))
    b = np.ascontiguousarray(bias.reshape(COUT, 1))
    in_maps = []
    for c in range(N_CORES):
        shard = _make_dual(
            input_tensor[c * IMGS_PER_CORE : (c + 1) * IMGS_PER_CORE]
        ).astype(hdt)
        in_maps.append({"x": shard, "wb": wb, "b": b})
    return in_maps


def _gather(results):
    return np.concatenate([results[c]["out"] for c in range(N_CORES)], axis=0)


def kernel(input_tensor, weights, bias):
    from concourse.bass_utils import run_bass_kernel_spmd

    nc = _get_nc()
    in_maps = _prepare_in_maps(input_tensor, weights, bias)
    res = run_bass_kernel_spmd(nc, in_maps, core_ids=list(range(N_CORES)))
    return _gather(res.results)
